# revision 2
# baseline (speedup 1.0000x reference)
"""3-layer GAT + graph pooling + MLP on 8 Trainium2 NeuronCores (Bass).

v2 design (edge phase is SWDGE-descgen bound, so minimize per-edge
descriptors and move everything else off GpSimd):

Sharding: core c owns dst-nodes [c*NLOC, (c+1)*NLOC) and their in-edges.
Per layer l:
  rows:   per OWN tile (128 nodes): psum_h = x_tile^T @ [W_l | W_l@A_l]
          -> bf16 row [h(192) | s_src(4) | pad] written to xtab slice;
          s_dst kept in SBUF (stab_sb).  AllGather slices -> xtab_full
          [GSLOTS, 256] bf16 (512B rows, gather-ready).
  edge:   one dma_gather of bf16 rows per (2-tile group, src-region),
          queues round-robined %4 so descgen overlaps ring drain
          (512B/edge; descgen ~5ns/idx is the bottleneck, so s_dst is
          NOT gathered).  Host streams one-hot matrices in both
          orientations (fp8): per 128-edge column q,
            t[e,4]   = ohT_q^T @ stab_tile        (per-edge s_dst)
            psum    += oh_q^T  @ [w*h | w]        (aggregation)
          with w = exp(leaky_relu(s_src + t)) built on DVE in bf16.
  post:   divide by summed w, +bias, ELU (f32); layers 1-2 feed the next
          layer's row phase; layer 3 feeds pooling (sum via one-hot
          matmul, max via offset scatter + window reduce) + MLP.
All data-dependent structure (indices, one-hots) is INPUT DATA; the
program is static and identical across cores (SPMD).  dma_scatter_add
is used only with unique indices (it is not collision-safe on HW).
"""

import sys
import numpy as np
import ml_dtypes

sys.path.insert(0, "/opt/trn_rl_repo")

BF = ml_dtypes.bfloat16
H, C = 4, 48
HC = H * C          # 192
NEG = 0.2
BIGNEG = -2.0e30
MAXOFF = 1000.0     # max-pool offset: x3 = elu(...) >= -1, so x3+1000 > 0
ROW = 256           # xtab row (bf16): h 192 | s_src 4 | pad -> 512B
REG = 32768         # int16 gather region size


def make_cfg(N=50000, E=800000, G=64, NC=8, FEAT=128, WIN=64):
    NLOC = N // NC
    assert NLOC * NC == N
    NSL = ((NLOC + 127) // 128) * 128
    return dict(N=N, E=E, G=G, NC=NC, FEAT=FEAT, NLOC=NLOC, NSL=NSL,
                GSLOTS=NC * NSL, WIN=WIN)


def _wrap_idx(idx):
    """SWDGE idx layout: element i -> [i % 16, i // 16], replicated to 128
    partitions (one copy per Q7 core)."""
    T = idx.shape[0]
    out = np.ascontiguousarray(idx.reshape(T // 16, 16).T).astype(np.int16)
    return np.tile(out, (8, 1))


def host_prep(cfg, adj, batch):
    N, G, NC = cfg["N"], cfg["G"], cfg["NC"]
    NLOC, NSL, WIN = cfg["NLOC"], cfg["NSL"], cfg["WIN"]
    NTL = NSL // 128
    src = np.asarray(adj[0], dtype=np.int64)
    dst = np.asarray(adj[1], dtype=np.int64)
    batch = np.asarray(batch, dtype=np.int64)
    # slot layout matches the two half-AllGathers: lower halves of all
    # cores at rows [0, NC*HALFR), upper halves after (both contiguous)
    HALFR = (NTL // 2) * 128
    UPR = NSL - HALFR
    sc_, sn_ = src // NLOC, src % NLOC
    src_slot = np.where(sn_ < HALFR, sc_ * HALFR + sn_,
                        NC * HALFR + sc_ * UPR + (sn_ - HALFR))
    nreg = (cfg["GSLOTS"] + REG - 1) // REG
    counts_g = np.bincount(batch, minlength=G)

    # ---- edge grouping: (dst-tile, src-region) blocks, each padded %128;
    # edges sorted by src_slot within a block (ascending gather addresses)
    blocks_all = []
    sizes = np.zeros((NC, NTL, nreg), dtype=np.int64)
    for c in range(NC):
        lo = c * NLOC
        esel = np.nonzero((dst >= lo) & (dst < lo + NLOC))[0]
        dt_of = (dst[esel] - lo) // 128
        rg_of = src_slot[esel] // REG
        d = {}
        for t in range(NTL):
            for r in range(nreg):
                ee = esel[(dt_of == t) & (rg_of == r)]
                ee = ee[np.argsort(src_slot[ee], kind="stable")]
                d[(t, r)] = ee
                sizes[c, t, r] = len(ee)
        blocks_all.append(d)
    bsz = np.zeros((NTL, nreg), dtype=np.int64)
    for t in range(NTL):
        for r in range(nreg):
            m = int(sizes[:, t, r].max())
            if r == 0:
                m = max(m, 1)          # ensure >=1 tile so psum gets reset
            bsz[t, r] = -(-m // 128) * 128 if m else 0
    # group-of-GK-tiles, region-major layout: one gather per (group, region)
    GK = 2
    groups = [list(range(g, min(g + GK, NTL))) for g in range(0, NTL, GK)]
    offs = np.zeros((NTL, nreg), dtype=np.int64)
    o = 0
    for tiles in groups:
        for r in range(nreg):
            for t in tiles:
                offs[t, r] = o
                o += bsz[t, r]
    TOT = int(o)
    cfg["groups"] = groups

    # ---- layer-3 graph-aligned slots
    pad3_meta, pad3_tot = [], 0
    for c in range(NC):
        lo = c * NLOC
        b = batch[lo:lo + NLOC]
        gids, starts = np.unique(b, return_index=True)
        osort = np.argsort(starts)
        gids, starts = gids[osort], starts[osort]
        ends = np.append(starts[1:], NLOC)
        slots = np.empty(NLOC, dtype=np.int64)
        wg, fwin = [], []
        pos = 0
        for g, s, e in zip(gids, starts, ends):
            cnt = e - s
            slots[s:e] = pos + np.arange(cnt)
            nw = -(-cnt // WIN)
            wg += [int(g)] * nw
            fwin += [1] + [0] * (nw - 1)
            pos += nw * WIN
        pad3_meta.append((slots, wg, fwin))
        pad3_tot = max(pad3_tot, pos)
    PAD3 = -(-pad3_tot // 128) * 128
    NW, NT3 = PAD3 // WIN, PAD3 // 128
    assert NW <= 128
    cfg.update(TOT=TOT, bsz=bsz, offs=offs, PAD3=PAD3, NW=NW, NT3=NT3,
               nreg=nreg, NTL=NTL)

    data = []
    strides = [1, 2, 4, 8, 16, 32]
    F8 = ml_dtypes.float8_e4m3
    ONE = np.float32(1.0).astype(F8).view(np.uint8)
    for c in range(NC):
        lo = c * NLOC
        slots3, wg, fwin = pad3_meta[c]
        g1 = np.zeros(TOT, dtype=np.int64)
        # one-hots, both orientations, partition-major bf16 (as uint16):
        # ohp [p=edge-in-col, col*128 + dstloc], ohTp [p=dstloc, col*128 + e]
        ohp = np.zeros((128, TOT), dtype=np.uint8)
        ohTp = np.zeros((128, TOT), dtype=np.uint8)
        for t in range(NTL):
            for r in range(nreg):
                ee = blocks_all[c][(t, r)]
                ne = len(ee)
                if ne == 0:
                    continue
                i0 = int(offs[t, r])
                g1[i0:i0 + ne] = src_slot[ee] - r * REG
                dloc = (dst[ee] - lo) - t * 128
                j = np.arange(ne)
                colj = (i0 + j) // 128
                ej = (i0 + j) % 128
                ohp[ej, colj * 128 + dloc] = ONE
                ohTp[dloc, colj * 128 + ej] = ONE
        assert g1.max() < REG
        s3 = np.full(NSL, PAD3, dtype=np.int64)
        s3[:NLOC] = slots3
        wgp = np.full(NW, -1, dtype=np.int64)
        wgp[:len(wg)] = wg
        cmb = np.full((128, len(strides)), BIGNEG, dtype=np.float32)
        for k, s in enumerate(strides):
            for i in range(NW - s):
                if wgp[i] >= 0 and wgp[i] == wgp[i + s]:
                    cmb[i, k] = 0.0
        wplace = np.full(128, G, dtype=np.int64)
        for i in range(len(wg)):
            if fwin[i]:
                wplace[i] = wg[i]
        onehot = np.zeros((NTL, 128, G), dtype=np.float32)
        nn = np.arange(NLOC)
        onehot[nn // 128, nn % 128, batch[lo:lo + NLOC]] = 1.0
        data.append(dict(
            g1=_wrap_idx(g1),
            oh=ohp.view(F8),
            ohT=ohTp.view(F8),
            s3=_wrap_idx(s3),
            cmb=cmb,
            wplace=_wrap_idx(wplace),
            onehot=onehot,
        ))
    inv_cnt = np.tile((1.0 / np.maximum(counts_g, 1.0))
                      .astype(np.float32)[None, :], (96, 1))
    return data, inv_cnt


def prep_float_inputs(cfg, inputs):
    FEAT = cfg["FEAT"]
    f = {}
    for l in (1, 2, 3):
        W = np.asarray(inputs[f"W{l}"], np.float32)
        A = np.zeros((HC, 2 * H), np.float32)
        for h in range(H):
            A[h * C:(h + 1) * C, h] = np.asarray(inputs[f"a_src{l}"], np.float32)[h]
            A[h * C:(h + 1) * C, H + h] = np.asarray(inputs[f"a_dst{l}"], np.float32)[h]
        f[f"Waug{l}"] = np.concatenate([W, W @ A], axis=1).astype(BF)
        f[f"brep{l}"] = np.tile(np.asarray(inputs[f"b{l}"], np.float32)[None, :],
                                (128, 1))
    f["fc1_w"] = np.asarray(inputs["fc1_w"], np.float32)
    f["fc1_b"] = np.asarray(inputs["fc1_b"], np.float32).reshape(-1, 1)
    f["out_w"] = np.asarray(inputs["out_w"], np.float32)
    f["out_b"] = np.asarray(inputs["out_b"], np.float32).reshape(-1, 1)
    return f


def build_program(cfg):
    from concourse import bacc, bass, mybir, tile
    from concourse.masks import make_identity
    f32, bf16, i16 = mybir.dt.float32, mybir.dt.bfloat16, mybir.dt.int16
    AF, ALU = mybir.ActivationFunctionType, mybir.AluOpType
    G, NC, FEAT = cfg["G"], cfg["NC"], cfg["FEAT"]
    NLOC, NSL, GSLOTS = cfg["NLOC"], cfg["NSL"], cfg["GSLOTS"]
    TOT, PAD3 = cfg["TOT"], cfg["PAD3"]
    NW, NT3, WIN, NTL = cfg["NW"], cfg["NT3"], cfg["WIN"], cfg["NTL"]
    NPW = 128 // WIN
    bsz, offs, nreg = cfg["bsz"], cfg["offs"], cfg["nreg"]
    SMAX = int(bsz.max()) // 128
    core_ids = list(range(NC))

    nc = bacc.Bacc(None, num_devices=NC, num_swdge_queues=4)

    featTo = nc.declare_dram_parameter("featTown", [FEAT, NSL], bf16, False)
    Waug, brep = [], []
    for l in (1, 2, 3):
        Waug.append(nc.declare_dram_parameter(
            f"Waug{l}", [FEAT if l == 1 else HC, HC + 2 * H], bf16, False))
        brep.append(nc.declare_dram_parameter(f"brep{l}", [128, HC], f32, False))
    fc1_w = nc.declare_dram_parameter("fc1_w", [2 * HC, 48], f32, False)
    fc1_b = nc.declare_dram_parameter("fc1_b", [48, 1], f32, False)
    out_w = nc.declare_dram_parameter("out_w", [48, 2], f32, False)
    out_b = nc.declare_dram_parameter("out_b", [2, 1], f32, False)
    inv_cnt = nc.declare_dram_parameter("inv_cnt", [96, G], f32, False)
    fp8 = mybir.dt.float8e4
    g1i = nc.declare_dram_parameter("g1", [128, TOT // 16], i16, False)
    ohi = nc.declare_dram_parameter("oh", [128, TOT], fp8, False)
    ohTi = nc.declare_dram_parameter("ohT", [128, TOT], fp8, False)
    s3i = nc.declare_dram_parameter("s3", [128, NSL // 16], i16, False)
    cmbi = nc.declare_dram_parameter("cmb", [128, 6], f32, False)
    wplacei = nc.declare_dram_parameter("wplace", [128, 8], i16, False)
    onehoti = nc.declare_dram_parameter("onehot", [NTL, 128, G], f32, False)
    yout = nc.declare_dram_parameter("y", [2, G], f32, True)

    xslice = [nc.dram_tensor(f"xslice{l}", [NSL, ROW], bf16) for l in range(3)]
    xfull = [nc.dram_tensor(f"xfull{l}", [GSLOTS, ROW], bf16,
                            addr_space="Shared") for l in range(3)]
    padgrid = nc.dram_tensor("padgrid", [PAD3 + 128, HC], f32)
    maxgrid = nc.dram_tensor("maxgrid", [G + 1, HC], f32)
    poolsl = nc.dram_tensor("poolsl", [96, 4, G], f32)
    poolag = nc.dram_tensor("poolag", [NC, 96, 4, G], f32, addr_space="Shared")

    with tile.TileContext(nc) as tc:
        with (
            tc.tile_pool(name="const", bufs=1) as constp,
            tc.tile_pool(name="wpool", bufs=1) as wpool,
            tc.tile_pool(name="row", bufs=2) as rowp,
            tc.tile_pool(name="edge", bufs=3) as edgep,
            tc.tile_pool(name="post", bufs=2) as postp,
            tc.tile_pool(name="psA", bufs=3, space="PSUM") as psA,
            tc.tile_pool(name="psT", bufs=2, space="PSUM") as psT,
            tc.tile_pool(name="psB", bufs=2, space="PSUM") as psB,
            tc.tile_pool(name="small", bufs=2) as smallp,
        ):
            ident = constp.tile([128, 128], f32)
            make_identity(nc, ident[:])

            wtA, wtB, bt = [], [], []
            for l in range(3):
                ka = FEAT if l == 0 else 96
                a = wpool.tile([ka, HC + 2 * H], bf16, tag=f"wtA{l}")
                nc.sync.dma_start(a[:], Waug[l][:ka])
                wtA.append(a)
                if l == 0:
                    wtB.append(None)
                else:
                    b_ = wpool.tile([96, HC + 2 * H], bf16, tag=f"wtB{l}")
                    nc.sync.dma_start(b_[:], Waug[l][96:])
                    wtB.append(b_)
                bb = wpool.tile([128, HC], f32, tag=f"bt{l}")
                nc.sync.dma_start(bb[:], brep[l][:])
                bt.append(bb)
            idxt = {}
            for nm, dram, w_ in (("g1", g1i, TOT // 16), ("s3", s3i, NSL // 16)):
                t = wpool.tile([128, w_], i16, tag=f"ix{nm}")
                nc.sync.dma_start(t[:], dram[:])
                idxt[nm] = t
            cmbt = wpool.tile([128, 6], f32, tag="cmb")
            nc.sync.dma_start(cmbt[:], cmbi[:])
            wplt = wpool.tile([128, 8], i16, tag="wpl")
            nc.sync.dma_start(wplt[:], wplacei[:])
            invt = wpool.tile([96, G], f32, tag="inv")
            nc.sync.dma_start(invt[:], inv_cnt[:])
            fc1wt = []
            for k in range(4):
                t = wpool.tile([96, 48], f32, tag=f"fc1{k}")
                nc.sync.dma_start(t[:], fc1_w[k * 96:(k + 1) * 96])
                fc1wt.append(t)
            fc1bt = wpool.tile([48, 1], f32, tag="fc1b")
            nc.sync.dma_start(fc1bt[:], fc1_b[:])
            outwt = wpool.tile([48, 2], f32, tag="outw")
            nc.sync.dma_start(outwt[:], out_w[:])
            outbt = wpool.tile([2, 1], f32, tag="outb")
            nc.sync.dma_start(outbt[:], out_b[:])

            featTowns = wpool.tile([FEAT, NSL], bf16, tag="fTo")
            nc.sync.dma_start(featTowns[:], featTo[:])
            negone = wpool.tile([128, 1], f32, tag="negone")
            nc.vector.memset(negone[:], -1.0)
            moff = wpool.tile([128, 1], f32, tag="moff")
            nc.vector.memset(moff[:], MAXOFF)
            # s_dst for own nodes, current layer (updated in place per tile)
            stab_sb = wpool.tile([128, NTL, H], bf16, tag="stab")

            zt = constp.tile([128, 16, HC], f32, tag="zt")
            nc.vector.memset(zt[:], 0.0)
            r0 = 0
            while r0 < PAD3 + 128:
                rr = min(2048, PAD3 + 128 - r0)
                nc.sync.dma_start(
                    padgrid[r0:r0 + rr].rearrange("(p a) c -> p (a c)", p=128),
                    zt[:, :rr // 128].rearrange("p a c -> p (a c)"))
                r0 += rr

            def row_tile(l, t, ytr):
                """Own-tile row phase for layer l+1 (l = producing layer idx,
                0 = from features).  ytr: None (use features) or [2][96,128]
                bf16 channel-major chunks.  Writes xtab row + stab_sb."""
                ps = psB.tile([128, HC + 2 * H], f32, tag="tps")
                if l == 0:
                    nc.tensor.matmul(ps[:], featTowns[:, t * 128:(t + 1) * 128],
                                     wtA[0][:], start=True, stop=True)
                else:
                    nc.tensor.matmul(ps[:], ytr[0][:], wtA[l][:],
                                     start=True, stop=False)
                    nc.tensor.matmul(ps[:], ytr[1][:], wtB[l][:],
                                     start=False, stop=True)
                row = rowp.tile([128, ROW], bf16, tag="xrow")
                nc.vector.tensor_copy(row[:, :HC + H], ps[:, :HC + H])
                nc.vector.tensor_copy(stab_sb[:, t], ps[:, HC + H:])
                nc.sync.dma_start(xslice[l][t * 128:(t + 1) * 128], row[:])

            def edge_group(l, tiles, consume, pre=None, gi=0):
                """Fetch all (t in tiles, r) edge blocks with one gather per
                region, then process tile-major.  consume(t, ps) drains the
                per-tile psum.  pre() runs right after the gathers are issued
                (deferred GpSimd work that must not block the gather stream)."""
                bigs, ohs, ohTs = {}, {}, {}
                for r in range(nreg):
                    Sg = sum(int(bsz[t, r]) for t in tiles)
                    if Sg == 0:
                        continue
                    i0g = int(offs[tiles[0], r])
                    ncg = Sg // 128
                    big = edgep.tile([128, ncg, ROW], bf16,
                                     tag=f"big{r}", bufs=4)
                    nc.gpsimd.dma_gather(
                        big[:],
                        xfull[l][r * REG:min(GSLOTS, (r + 1) * REG)],
                        idxt["g1"][:, i0g // 16:(i0g + Sg) // 16], Sg, Sg, ROW,
                        single_packet=False,
                        queue_num=(gi * nreg + r) % 4)
                    oht_ = edgep.tile([128, ncg, 128], fp8,
                                      tag=f"ohT{r}", bufs=4)
                    nc.sync.dma_start(
                        oht_[:],
                        ohTi[:, i0g:i0g + Sg].rearrange(
                            "p (a d) -> p a d", d=128))
                    oh_ = edgep.tile([128, ncg, 128], fp8,
                                     tag=f"oh{r}", bufs=4)
                    nc.scalar.dma_start(
                        oh_[:],
                        ohi[:, i0g:i0g + Sg].rearrange(
                            "p (a d) -> p a d", d=128))
                    bigs[r], ohs[r], ohTs[r] = big, oh_, oht_
                if pre is not None:
                    pre()
                for t in tiles:
                    ps = psA.tile([128, HC + H], f32, tag="agg")
                    first = True
                    nblk = sum(1 for r in range(nreg) if bsz[t, r] > 0)
                    bi = 0
                    for r in range(nreg):
                        S = int(bsz[t, r])
                        if S == 0:
                            continue
                        bi += 1
                        ncols = S // 128
                        lo = (int(offs[t, r]) - int(offs[tiles[0], r])) // 128
                        big, oh_, oht_ = bigs[r], ohs[r], ohTs[r]
                        tps = psT.tile([128, SMAX, H], f32, tag="tsc")
                        for q in range(ncols):
                            nc.tensor.matmul(
                                tps[:, q], oht_[:, lo + q], stab_sb[:, t],
                                start=(q == 0), stop=(q == ncols - 1),
                                skip_group_check=True)
                        tcol = edgep.tile([128, SMAX, H], bf16, tag="tcol")
                        nc.scalar.copy(
                            tcol[:, :ncols].rearrange("p a h -> p (a h)"),
                            tps[:, :ncols].rearrange("p a h -> p (a h)"))
                        w = edgep.tile([128, SMAX, H], bf16, tag="w", bufs=3)
                        nc.vector.tensor_add(
                            w[:, :ncols],
                            big[:, lo:lo + ncols, HC:HC + H],
                            tcol[:, :ncols])
                        nc.scalar.activation(
                            w[:, :ncols].rearrange("p a h -> p (a h)"),
                            w[:, :ncols].rearrange("p a h -> p (a h)"),
                            AF.Prelu, alpha=NEG)
                        nc.scalar.activation(
                            w[:, :ncols].rearrange("p a h -> p (a h)"),
                            w[:, :ncols].rearrange("p a h -> p (a h)"), AF.Exp)
                        sc = edgep.tile([128, SMAX, HC + H], bf16,
                                        tag="sc", bufs=3)
                        nc.vector.tensor_mul(
                            sc[:, :ncols, :HC].rearrange(
                                "p a (h c) -> p a h c", c=C),
                            big[:, lo:lo + ncols, :HC].rearrange(
                                "p a (h c) -> p a h c", c=C),
                            w[:, :ncols].rearrange("p a (h o) -> p a h o", o=1)
                            .to_broadcast([128, ncols, H, C]))
                        nc.scalar.copy(sc[:, :ncols, HC:], w[:, :ncols])
                        for q in range(ncols):
                            nc.tensor.matmul(
                                ps[:, :HC + H], oh_[:, lo + q], sc[:, q],
                                start=first,
                                stop=(bi == nblk and q == ncols - 1))
                            first = False
                    consume(t, ps)

            def post_tile(l, ps):
                den = postp.tile([128, H], f32, tag="pden")
                nc.vector.tensor_scalar(den[:], ps[:, HC:HC + H], 1e-16,
                                        None, ALU.max)
                nc.vector.reciprocal(den[:], den[:])
                y = postp.tile([128, HC], f32, tag="py", bufs=6)
                nc.vector.tensor_mul(
                    y[:].rearrange("p (h c) -> p h c", c=C),
                    ps[:, :HC].rearrange("p (h c) -> p h c", c=C),
                    den[:].rearrange("p (h o) -> p h o", o=1)
                    .to_broadcast([128, H, C]))
                nc.vector.tensor_add(y[:], y[:], bt[l][:])
                e = postp.tile([128, HC], f32, tag="pe")
                # ELU: e = exp(min(y,0))-1 = exp(-relu(-y))-1, then max(y, e)
                nc.scalar.activation(e[:], y[:], AF.Relu, scale=-1.0)
                nc.scalar.activation(e[:], e[:], AF.Exp, scale=-1.0)
                nc.scalar.activation(e[:], e[:], AF.Identity, bias=negone[:])
                nc.vector.tensor_max(y[:], y[:], e[:])
                return y

            def transpose_y(y):
                """y [128, 192] f32 -> two [96, 128] bf16 channel-major."""
                ytr = []
                for blk in range(2):
                    pt = psB.tile([96, 128], f32, tag="tps")
                    nc.tensor.transpose(
                        pt[:], y[:, blk * 96:(blk + 1) * 96], ident[:])
                    yb = rowp.tile([96, 128], bf16, tag=f"ytr{blk}")
                    nc.vector.tensor_copy(yb[:], pt[:])
                    ytr.append(yb)
                return ytr

            HALF = (NTL // 2) * 128

            def half_gather(lx, h):
                h0, h1 = (0, HALF) if h == 0 else (HALF, NSL)
                o0, o1 = (0, NC * HALF) if h == 0 else (NC * HALF, GSLOTS)
                nc.gpsimd.collective_compute(
                    "AllGather", mybir.AluOpType.bypass,
                    replica_groups=[core_ids],
                    ins=[xslice[lx][h0:h1]], outs=[xfull[lx][o0:o1]])

            # layer-1 rows from features
            for t in range(NTL):
                row_tile(0, t, None)
                if (t + 1) * 128 == HALF:
                    half_gather(0, 0)
            half_gather(0, 1)

            for l in range(3):
                if l < 2:
                    deferred = []
                    pend_y = []

                    def finish_mid(t, y, _l=l, _d=deferred):
                        ytr = transpose_y(y)
                        row_tile(_l + 1, t, ytr)
                        if (t + 1) * 128 == HALF:
                            _d.append(lambda: half_gather(_l + 1, 0))

                    def consume_mid(t, ps, _l=l, _p=pend_y):
                        y = post_tile(_l, ps)
                        _p.append((t, y))
                        if len(_p) > 3:
                            finish_mid(*_p.pop(0))

                    def pre_mid(_d=deferred):
                        while _d:
                            _d.pop(0)()
                    for gi, tiles in enumerate(cfg["groups"]):
                        edge_group(l, tiles, consume_mid, pre=pre_mid, gi=gi)
                    while pend_y:
                        finish_mid(*pend_y.pop(0))
                    pre_mid()
                    half_gather(l + 1, 1)
                else:
                    sumacc = wpool.tile([96, 2, G], f32, tag="sumacc")
                    nc.vector.memset(sumacc[:], 0.0)

                    pend_sc = []
                    pend_y3 = []

                    def finish_pool(t, y):
                        oht = smallp.tile([128, G], f32, tag="oh2")
                        nc.sync.dma_start(oht[:], onehoti[t])
                        for blk in range(2):
                            sp = psB.tile([96, G], f32, tag="tps")
                            nc.tensor.matmul(
                                sp[:], y[:, blk * 96:(blk + 1) * 96],
                                oht[:], start=True, stop=True)
                            nc.vector.tensor_add(sumacc[:, blk], sumacc[:, blk],
                                                 sp[:])
                        yo = postp.tile([128, HC], f32, tag="pyo", bufs=6)
                        nc.scalar.activation(yo[:], y[:], AF.Identity,
                                             bias=moff[:])
                        pend_sc.append((t, yo))

                    def consume_pool(t, ps):
                        y = post_tile(2, ps)
                        pend_y3.append((t, y))
                        if len(pend_y3) > 3:
                            finish_pool(*pend_y3.pop(0))

                    def flush_sc():
                        while pend_sc:
                            t, yo = pend_sc.pop(0)
                            nc.gpsimd.dma_scatter_add(
                                padgrid[:, :],
                                yo[:].rearrange("p (a c) -> p a c", a=1),
                                idxt["s3"][:, t * 8:(t + 1) * 8], 128, 128, HC,
                                single_packet=False)

                    for gi, tiles in enumerate(cfg["groups"]):
                        edge_group(l, tiles, consume_pool, pre=flush_sc,
                                   gi=gi)
                    while pend_y3:
                        finish_pool(*pend_y3.pop(0))
                    flush_sc()
                    wmax = smallp.tile([96, 2, NW], f32, tag="wmax")
                    for t in range(NT3):
                        rows = postp.tile([128, HC], f32, tag="prow3")
                        nc.sync.dma_start(rows[:],
                                          padgrid[t * 128:(t + 1) * 128])
                        for blk in range(2):
                            pt = psB.tile([96, 128], f32, tag="tps")
                            nc.tensor.transpose(
                                pt[:], rows[:, blk * 96:(blk + 1) * 96],
                                ident[:])
                            nc.vector.tensor_reduce(
                                wmax[:, blk, t * NPW:(t + 1) * NPW],
                                pt[:].rearrange("p (w q) -> p w q", q=WIN),
                                mybir.AxisListType.X, ALU.max)
                    wrow = smallp.tile([128, HC], f32, tag="wrow")
                    for blk in range(2):
                        pt2 = psB.tile([128, 96], f32, tag="tps")
                        nc.tensor.transpose(pt2[:NW], wmax[:, blk],
                                            ident[:96, :96])
                        nc.vector.tensor_copy(
                            wrow[:NW, blk * 96:(blk + 1) * 96], pt2[:NW])
                    for ki, s in enumerate([1, 2, 4, 8, 16, 32]):
                        if s >= NW:
                            break
                        sh = smallp.tile([128, HC], f32, tag="wsh")
                        nc.sync.dma_start(sh[:NW - s], wrow[s:NW])
                        nc.vector.tensor_scalar(sh[:NW - s], sh[:NW - s],
                                                cmbt[:NW - s, ki:ki + 1],
                                                None, ALU.add)
                        nc.vector.tensor_max(wrow[:NW - s], wrow[:NW - s],
                                             sh[:NW - s])
                    zg = smallp.tile([G + 1, HC], f32, tag="zg")
                    nc.vector.memset(zg[:], 0.0)
                    nc.sync.dma_start(maxgrid[:], zg[:])
                    nc.gpsimd.dma_scatter_add(
                        maxgrid[:], wrow[:].rearrange("p (a c) -> p a c", a=1),
                        wplt[:], 128, 128, HC, single_packet=False)
                    mg = smallp.tile([G, HC], f32, tag="mg")
                    nc.sync.dma_start(mg[:], maxgrid[:G])
                    pp = smallp.tile([96, 4, G], f32, tag="pp")
                    for blk in range(2):
                        nc.vector.tensor_copy(pp[:, blk], sumacc[:, blk])
                        pt3 = psB.tile([96, G], f32, tag="tps")
                        nc.tensor.transpose(
                            pt3[:], mg[:, blk * 96:(blk + 1) * 96],
                            ident[:G, :G])
                        nc.vector.tensor_copy(pp[:, 2 + blk], pt3[:])
                    nc.sync.dma_start(poolsl[:], pp[:])
                    nc.gpsimd.collective_compute(
                        "AllGather", mybir.AluOpType.bypass,
                        replica_groups=[core_ids],
                        ins=[poolsl[:]], outs=[poolag[:]])
                    agg = smallp.tile([96, 4, G], f32, tag="agg2")
                    for c_ in range(NC):
                        at = smallp.tile([96, 4, G], f32, tag="agt")
                        nc.sync.dma_start(at[:], poolag[c_])
                        if c_ == 0:
                            nc.vector.tensor_copy(agg[:], at[:])
                        else:
                            nc.vector.tensor_add(agg[:, :2], agg[:, :2],
                                                 at[:, :2])
                            nc.vector.tensor_max(agg[:, 2:], agg[:, 2:],
                                                 at[:, 2:])
                    for blk in range(2):
                        nc.vector.tensor_mul(agg[:, blk], agg[:, blk], invt[:])
                        nc.vector.tensor_scalar(agg[:, 2 + blk],
                                                agg[:, 2 + blk],
                                                -MAXOFF, None, ALU.add)
                    zp = psB.tile([48, G], f32, tag="tps")
                    for k in range(4):
                        nc.tensor.matmul(zp[:], fc1wt[k], agg[:, k],
                                         start=(k == 0), stop=(k == 3))
                    z = smallp.tile([48, G], f32, tag="z")
                    nc.vector.tensor_scalar(z[:], zp[:], fc1bt[:], None,
                                            ALU.add)
                    e2 = smallp.tile([48, G], f32, tag="e2")
                    nc.vector.tensor_scalar(e2[:], z[:], 0.0, None, ALU.min)
                    nc.scalar.activation(e2[:], e2[:], AF.Exp)
                    nc.vector.tensor_scalar(e2[:], e2[:], -1.0, None, ALU.add)
                    nc.vector.tensor_max(z[:], z[:], e2[:])
                    yp = psB.tile([2, G], f32, tag="tps")
                    nc.tensor.matmul(yp[:], outwt[:], z[:], start=True,
                                     stop=True)
                    yf = smallp.tile([2, G], f32, tag="yf")
                    nc.vector.tensor_scalar(yf[:], yp[:], outbt[:], None,
                                            ALU.add)
                    nc.sync.dma_start(yout[:], yf[:])
    nc.finalize()
    return nc


def run(inputs, cfg, **run_kw):
    data, inv_cnt = host_prep(cfg, inputs["adj"], inputs["batch"])
    fl = prep_float_inputs(cfg, inputs)
    NC, NLOC, NSL = cfg["NC"], cfg["NLOC"], cfg["NSL"]
    in_maps = []
    feat = np.asarray(inputs["features"], np.float32)
    for c in range(NC):
        m = dict(fl)
        m["inv_cnt"] = inv_cnt
        fto = np.zeros((cfg["FEAT"], NSL), np.float32)
        fto[:, :NLOC] = feat[c * NLOC:(c + 1) * NLOC].T
        m["featTown"] = fto.astype(BF)
        m.update({k: data[c][k] for k in
                  ("g1", "oh", "ohT", "s3", "cmb", "wplace", "onehot")})
        in_maps.append(m)
    nc = build_program(cfg)
    from concourse.bass_utils import run_bass_kernel_spmd
    res = run_bass_kernel_spmd(nc, in_maps, list(range(NC)), **run_kw)
    y = np.asarray(res.results[0]["y"])
    return y.T.copy(), res


def kernel(**inputs):
    y, _ = run(inputs, make_cfg())
    return y
